# revision 6
# baseline (speedup 1.0000x reference)
import os
import sys

for _p in ("/opt/trn_rl_repo", "/root/.axon_site/_ro/trn_rl_repo"):
    if os.path.isdir(_p) and _p not in sys.path:
        sys.path.insert(0, _p)

import numpy as np
from concourse import bacc, tile, mybir
from concourse.bass_utils import run_bass_kernel_spmd

# Problem shapes (hardcoded per spec): x [32,1024,1024], W [3072,1024],
# bias [3072], A0/A1 [5,1024], B0/B1 [1024,5], s0/s1 scalar.
# out [32,1024,3072] = x @ (W + pad(cat(s0*B0@A0, s1*B1@A1)))^T + bias
# Sharding: data-parallel over batch, 4 batches (4096 tokens) per core.
#
# Per-core structure:
#   - W'^T = (W + delta)^T resident in SBUF as 48 bf16 tiles [128,512]
#     (PE transposes in fp32r, LoRA rank-5 accumulated in fp32 PSUM from
#     replicated A / (s*B)^T factors, single bf16 round on the DVE drain).
#   - x is PE-transposed per 512-token superchunk (fp32r), drained to bf16.
#   - Main matmuls run bf16 x bf16 -> fp32 PSUM at 1 cycle/row.
#   - W prep PSUM groups are interleaved 1:1 with the first two superchunks'
#     accumulation groups, keeping PE MAC activity high (HAM clock stays at
#     2.4 GHz) and hiding the 12 MB W DMA.
#   - DMA queues: x on scalar (HWDGE), W on gpsimd (SWDGE), consts + output
#     stores on sync (HWDGE).
#   - Host-side marshalling: (s*B).T is precomputed on host (tiny [1024,5]
#     tensors; avoids a pathological 4-byte-strided gather DMA on device).
B, S, D = 32, 1024, 1024
O = 3 * D
R = 5
N_CORES = 8
TOK = B * S // N_CORES          # 4096 tokens per core
P = 128
NO = 512                        # output free-dim chunk (one PSUM bank, fp32)
N_D = D // P                    # 8 contraction chunks
N_OC = O // NO                  # 6 output 512-blocks
N_SUP = TOK // NO               # 8 super chunks of 512 tokens
TC = NO // P                    # 4 token tiles per super chunk

F32 = mybir.dt.float32
F32R = mybir.dt.float32r
BF16 = mybir.dt.bfloat16

_CACHE = {}


def _build():
    nc = bacc.Bacc("TRN2", target_bir_lowering=False, debug=False,
                   num_devices=N_CORES)
    x_d = nc.declare_dram_parameter("x", [TOK, D], F32, isOutput=False)
    w_d = nc.declare_dram_parameter("w", [O, D], F32, isOutput=False)
    bias_d = nc.declare_dram_parameter("bias", [1, O], F32, isOutput=False)
    a0_d = nc.declare_dram_parameter("a0", [R, D], F32, isOutput=False)
    a1_d = nc.declare_dram_parameter("a1", [R, D], F32, isOutput=False)
    bt0_d = nc.declare_dram_parameter("bt0", [R, D], F32, isOutput=False)
    bt1_d = nc.declare_dram_parameter("bt1", [R, D], F32, isOutput=False)
    ident_d = nc.declare_dram_parameter("ident", [P, P], F32, isOutput=False)
    ones_d = nc.declare_dram_parameter("ones", [1, P], F32, isOutput=False)
    out_d = nc.declare_dram_parameter("out", [TOK, O], F32, isOutput=True)

    ADD = mybir.AluOpType.add

    with tile.TileContext(nc) as tc:
        with tc.tile_pool(name="const", bufs=1) as cpool, \
             tc.tile_pool(name="wres", bufs=1) as wpool, \
             tc.tile_pool(name="xload", bufs=8) as xpool, \
             tc.tile_pool(name="wload", bufs=4) as wnpool, \
             tc.tile_pool(name="xt", bufs=2) as xtpool, \
             tc.tile_pool(name="ostage", bufs=4) as opool, \
             tc.tile_pool(name="psA", bufs=4, space="PSUM") as psA, \
             tc.tile_pool(name="psT", bufs=4, space="PSUM") as psT:

            # ---- const DMAs (sync queue; ident first: transposes need it) ----
            ident_sb = cpool.tile([P, P], F32R, tag="ident")
            nc.sync.dma_start(out=ident_sb[:], in_=ident_d[:].bitcast(F32R))

            # ---- x superchunk 0/1 loads (scalar queue) ----
            def emit_x_loads(sp):
                # two half-row DMAs per tile so transposes of the first
                # 512 columns can start before the full tile lands
                x_nat = []
                for tci in range(TC):
                    row0 = sp * NO + tci * P
                    xn = xpool.tile([P, D], F32R, tag="xnat", name=f"xn{sp}_{tci}")
                    for h in range(2):
                        cs = slice(h * (D // 2), (h + 1) * (D // 2))
                        nc.scalar.dma_start(out=xn[:, cs],
                                            in_=x_d[row0:row0 + P, cs].bitcast(F32R))
                    x_nat.append(xn)
                return x_nat

            x_nat_pending = {0: emit_x_loads(0), 1: emit_x_loads(1)}

            # remaining consts on sync queue (all small + natural layout)
            ones_sb = cpool.tile([1, P], F32, tag="ones")
            nc.sync.dma_start(out=ones_sb[:], in_=ones_d[:])
            bias1_sb = cpool.tile([1, O], F32, tag="bias1")
            nc.sync.dma_start(out=bias1_sb[:], in_=bias_d[:])
            a_sb = []
            for i, ad in enumerate((a0_d, a1_d)):
                t = cpool.tile([R, D], F32R, tag=f"a{i}", name=f"a{i}")
                nc.sync.dma_start(out=t[:], in_=ad[:].bitcast(F32R))
                a_sb.append(t)
            bt_sb = []
            for i, bd in enumerate((bt0_d, bt1_d)):
                t = cpool.tile([R, D], F32R, tag=f"bt{i}", name=f"bt{i}")
                nc.sync.dma_start(out=t[:], in_=bd[:].bitcast(F32R))
                bt_sb.append(t)

            # ---- W loads (gpsimd queue, all 24 tiles; ring-buffered) ----
            w_nat = {}
            for ocb in range(N_OC):
                tiles = []
                for j in range(TC):
                    oc = ocb * TC + j
                    wn = wnpool.tile([P, D], F32R, tag="wnat", name=f"wn{oc}")
                    nc.gpsimd.dma_start(out=wn[:],
                                        in_=w_d[oc * P:(oc + 1) * P, :].bitcast(F32R))
                    tiles.append(wn)
                w_nat[ocb] = tiles

            # ---- x transpose per superchunk: fp32r PE transpose, bf16 drain ----
            def emit_x_transposes(sp, x_nat):
                xg = [[None, None] for _ in range(TC)]
                for tci in range(TC):
                    for g in range(2):
                        tp = psT.tile([P, NO], F32R, tag="tp", name="tp")
                        for k in range(4):
                            d = g * 4 + k
                            nc.tensor.matmul(tp[:, k * P:(k + 1) * P],
                                             x_nat[tci][:, d * P:(d + 1) * P],
                                             ident_sb[:], is_transpose=True,
                                             start=(k == 0), stop=(k == 3),
                                             skip_group_check=True)
                        xgt = xtpool.tile([P, NO], BF16, tag=f"xg{tci}_{g}",
                                          name=f"xg{tci}_{g}")
                        nc.vector.tensor_copy(xgt[:], tp[:].bitcast(F32))
                        xg[tci][g] = xgt
                return xg

            xg_pending = {0: emit_x_transposes(0, x_nat_pending.pop(0))}

            # ---- bias broadcast across partitions: [128, 3072] ----
            # (between the two transpose phases: PE filler while x DMA lands)
            bias_bc = cpool.tile([P, O], F32, tag="biasbc")
            for j in range(N_OC):
                sl = slice(j * NO, (j + 1) * NO)
                b_ps = psA.tile([P, NO], F32, tag="acc")
                nc.tensor.matmul(b_ps[:], ones_sb[:], bias1_sb[:, sl],
                                 start=True, stop=True)
                nc.vector.tensor_copy(bias_bc[:, sl], b_ps[:])

            xg_pending[1] = emit_x_transposes(1, x_nat_pending.pop(1))

            # ---- resident W'^T, 48 bf16 tiles [128, 512]: wt[d][ocb] ----
            wt = [[wpool.tile([P, NO], BF16, tag=f"wt{d}_{ocb}",
                              name=f"wt{d}_{ocb}")
                   for ocb in range(N_OC)] for d in range(N_D)]

            def emit_w_prep_group(ocb, d):
                # one PSUM group: 4 PE transposes (+ LoRA accumulate) + drain
                tp = psT.tile([P, NO], F32R, tag="tp")
                for j in range(TC):
                    nc.tensor.matmul(tp[:, j * P:(j + 1) * P],
                                     w_nat[ocb][j][:, d * P:(d + 1) * P],
                                     ident_sb[:], is_transpose=True,
                                     start=(j == 0),
                                     stop=(j == TC - 1 and ocb < 2),
                                     skip_group_check=True)
                if ocb >= 2:
                    f = 0 if ocb < 4 else 1
                    lo = ocb * NO - D - (D if f else 0)
                    nc.tensor.matmul(tp[:].bitcast(F32),
                                     a_sb[f][:, d * P:(d + 1) * P],
                                     bt_sb[f][:, lo:lo + NO],
                                     start=False, stop=True,
                                     skip_group_check=True)
                nc.vector.tensor_copy(wt[d][ocb][:], tp[:].bitcast(F32))

            # ---- one accumulation group of main matmuls + drain + store ----
            def emit_acc_group(sp, tci, oc, xg):
                trow = slice(sp * NO + tci * P, sp * NO + (tci + 1) * P)
                osl = slice(oc * NO, (oc + 1) * NO)
                acc = psA.tile([P, NO], F32, tag="acc", name="acc")
                for d in range(N_D):
                    lhsT = xg[tci][d // 4][:, (d % 4) * P:(d % 4 + 1) * P]
                    nc.tensor.matmul(acc[:], lhsT, wt[d][oc][:],
                                     start=(d == 0), stop=(d == N_D - 1))
                o_sb = opool.tile([P, NO], F32, tag="ost", name="ost")
                nc.vector.tensor_tensor(out=o_sb[:], in0=acc[:],
                                        in1=bias_bc[:, osl], op=ADD)
                nc.sync.dma_start(out=out_d[trow, osl], in_=o_sb[:])

            # ---- startup: W prep interleaved with sp0/sp1 matmuls ----
            # prep(0) first, then for each ocb: 8 acc groups (sp0/sp1 x 4 tci)
            # interleaved 1:1 with the 8 prep groups of ocb+1.
            for d in range(N_D):
                emit_w_prep_group(0, d)
            for ocb in range(N_OC):
                groups = [(sp, tci) for sp in (0, 1) for tci in range(TC)]
                for i, (sp, tci) in enumerate(groups):
                    emit_acc_group(sp, tci, ocb, xg_pending[sp])
                    if ocb + 1 < N_OC:
                        emit_w_prep_group(ocb + 1, i)

            # ---- steady state: superchunks 2..7 ----
            for sp in range(2, N_SUP):
                x_nat = emit_x_loads(sp)
                xg = emit_x_transposes(sp, x_nat)
                for tci in range(TC):
                    for oc in range(N_OC):
                        emit_acc_group(sp, tci, oc, xg)

    nc.compile()
    return nc


def kernel(x, W, bias, A0, A1, B0, B1, s0, s1, **run_kwargs):
    x = np.asarray(x, dtype=np.float32)
    if "nc" not in _CACHE:
        _CACHE["nc"] = _build()
    nc = _CACHE["nc"]

    s0 = np.float32(np.asarray(s0).reshape(()))
    s1 = np.float32(np.asarray(s1).reshape(()))
    shared = {
        "w": np.ascontiguousarray(np.asarray(W, np.float32)),
        "bias": np.asarray(bias, np.float32).reshape(1, O),
        "a0": np.ascontiguousarray(np.asarray(A0, np.float32)),
        "a1": np.ascontiguousarray(np.asarray(A1, np.float32)),
        "bt0": np.ascontiguousarray((s0 * np.asarray(B0, np.float32)).T),
        "bt1": np.ascontiguousarray((s1 * np.asarray(B1, np.float32)).T),
        "ident": np.eye(P, dtype=np.float32),
        "ones": np.ones((1, P), np.float32),
    }
    xr = x.reshape(N_CORES, TOK, D)
    in_maps = [{**shared, "x": np.ascontiguousarray(xr[c])} for c in range(N_CORES)]
    res = run_bass_kernel_spmd(nc, in_maps, list(range(N_CORES)), **run_kwargs)
    out = np.concatenate([res.results[c]["out"][None] for c in range(N_CORES)], 0)
    full = out.reshape(B, S, O)
    _CACHE["last_result"] = res
    return full


# revision 10
# speedup vs baseline: 1.0441x; 1.0441x over previous
import os
import sys

for _p in ("/opt/trn_rl_repo", "/root/.axon_site/_ro/trn_rl_repo"):
    if os.path.isdir(_p) and _p not in sys.path:
        sys.path.insert(0, _p)

import numpy as np
from concourse import bacc, tile, mybir
from concourse.bass_utils import run_bass_kernel_spmd

# Problem shapes (hardcoded per spec): x [32,1024,1024], W [3072,1024],
# bias [3072], A0/A1 [5,1024], B0/B1 [1024,5], s0/s1 scalar.
# out [32,1024,3072] = x @ (W + pad(cat(s0*B0@A0, s1*B1@A1)))^T + bias
# Sharding: data-parallel over batch, 4 batches (4096 tokens) per core.
#
# Per-core structure:
#   - W'^T = (W + delta)^T resident in SBUF as 48 bf16 tiles [128,512]
#     (PE transposes in fp32r, LoRA rank-5 accumulated in fp32 PSUM from
#     replicated A / (s*B)^T factors, single bf16 round on the DVE drain).
#   - x is PE-transposed per 512-token superchunk (fp32r), drained to bf16.
#   - Main matmuls run bf16 x bf16 -> fp32 PSUM at 1 cycle/row.
#   - W prep PSUM groups are interleaved 1:1 with the first two superchunks'
#     accumulation groups, keeping PE MAC activity high (HAM clock stays at
#     2.4 GHz) and hiding the 12 MB W DMA.
#   - DMA queues: x on scalar (HWDGE), W on gpsimd (SWDGE), consts + output
#     stores on sync (HWDGE).
#   - Host-side marshalling: (s*B).T is precomputed on host (tiny [1024,5]
#     tensors; avoids a pathological 4-byte-strided gather DMA on device).
B, S, D = 32, 1024, 1024
O = 3 * D
R = 5
N_CORES = 8
TOK = B * S // N_CORES          # 4096 tokens per core
P = 128
NO = 512                        # output free-dim chunk (one PSUM bank, fp32)
N_D = D // P                    # 8 contraction chunks
N_OC = O // NO                  # 6 output 512-blocks
N_SUP = TOK // NO               # 8 super chunks of 512 tokens
TC = NO // P                    # 4 token tiles per super chunk

F32 = mybir.dt.float32
F32R = mybir.dt.float32r
BF16 = mybir.dt.bfloat16

_CACHE = {}


def _build():
    nc = bacc.Bacc("TRN2", target_bir_lowering=False, debug=False,
                   num_devices=N_CORES)
    x_d = nc.declare_dram_parameter("x", [TOK, D], F32, isOutput=False)
    w_d = nc.declare_dram_parameter("w", [O, D], F32, isOutput=False)
    bias_d = nc.declare_dram_parameter("bias", [1, O], F32, isOutput=False)
    a0_d = nc.declare_dram_parameter("a0", [R, D], F32, isOutput=False)
    a1_d = nc.declare_dram_parameter("a1", [R, D], F32, isOutput=False)
    bt0_d = nc.declare_dram_parameter("bt0", [R, D], F32, isOutput=False)
    bt1_d = nc.declare_dram_parameter("bt1", [R, D], F32, isOutput=False)
    ident_d = nc.declare_dram_parameter("ident", [P, P], F32, isOutput=False)
    ones_d = nc.declare_dram_parameter("ones", [1, P], F32, isOutput=False)
    out_d = nc.declare_dram_parameter("out", [TOK, O], F32, isOutput=True)

    ADD = mybir.AluOpType.add

    with tile.TileContext(nc) as tc:
        with tc.tile_pool(name="const", bufs=1) as cpool, \
             tc.tile_pool(name="wres", bufs=1) as wpool, \
             tc.tile_pool(name="xload", bufs=8) as xpool, \
             tc.tile_pool(name="wload", bufs=8) as wnpool, \
             tc.tile_pool(name="xt", bufs=2) as xtpool, \
             tc.tile_pool(name="ostage", bufs=4) as opool, \
             tc.tile_pool(name="psA", bufs=4, space="PSUM") as psA, \
             tc.tile_pool(name="psT", bufs=4, space="PSUM") as psT:

            # ---- const DMAs (sync queue; ident first: transposes need it) ----
            ident_sb = cpool.tile([P, P], F32R, tag="ident")
            nc.sync.dma_start(out=ident_sb[:], in_=ident_d[:].bitcast(F32R))

            # ---- x superchunk 0/1 loads (scalar queue) ----
            def emit_x_loads(sp):
                x_nat = []
                for tci in range(TC):
                    row0 = sp * NO + tci * P
                    xn = xpool.tile([P, D], F32R, tag="xnat", name=f"xn{sp}_{tci}")
                    nc.scalar.dma_start(out=xn[:],
                                        in_=x_d[row0:row0 + P, :].bitcast(F32R))
                    x_nat.append(xn)
                return x_nat

            x_nat_pending = {0: emit_x_loads(0), 1: emit_x_loads(1)}

            # remaining consts on sync queue (all small + natural layout)
            ones_sb = cpool.tile([1, P], F32, tag="ones")
            nc.sync.dma_start(out=ones_sb[:], in_=ones_d[:])
            bias1_sb = cpool.tile([1, O], F32, tag="bias1")
            nc.sync.dma_start(out=bias1_sb[:], in_=bias_d[:])
            a_sb = []
            for i, ad in enumerate((a0_d, a1_d)):
                t = cpool.tile([R, D], F32R, tag=f"a{i}", name=f"a{i}")
                nc.sync.dma_start(out=t[:], in_=ad[:].bitcast(F32R))
                a_sb.append(t)
            bt_sb = []
            for i, bd in enumerate((bt0_d, bt1_d)):
                t = cpool.tile([R, D], F32R, tag=f"bt{i}", name=f"bt{i}")
                nc.sync.dma_start(out=t[:], in_=bd[:].bitcast(F32R))
                bt_sb.append(t)

            # ---- W loads (gpsimd queue, all 24 tiles; ring-buffered) ----
            # Gate the W DMA flood behind sp0's x arrival: a tiny gpsimd read
            # of the last sp0 x tile makes the gpsimd queue wait, giving the
            # latency-critical x loads the HBM to themselves early on. W has
            # ~80us of slack before prep consumes the later tiles.
            gate_sb = cpool.tile([1, 2], F32, tag="gate")
            nc.gpsimd.tensor_copy(gate_sb[:],
                                  x_nat_pending[0][TC - 1][0:1, 0:2].bitcast(F32))
            w_nat = {}
            for ocb in range(N_OC):
                tiles = []
                for j in range(TC):
                    oc = ocb * TC + j
                    wn = wnpool.tile([P, D], F32R, tag="wnat", name=f"wn{oc}")
                    nc.gpsimd.dma_start(out=wn[:],
                                        in_=w_d[oc * P:(oc + 1) * P, :].bitcast(F32R))
                    tiles.append(wn)
                w_nat[ocb] = tiles

            # ---- x transpose per superchunk: fp32r PE transpose, bf16 drain ----
            def emit_x_transposes(sp, x_nat):
                xg = [[None, None] for _ in range(TC)]
                for tci in range(TC):
                    for g in range(2):
                        tp = psT.tile([P, NO], F32R, tag="tp", name="tp")
                        for k in range(4):
                            d = g * 4 + k
                            nc.tensor.matmul(tp[:, k * P:(k + 1) * P],
                                             x_nat[tci][:, d * P:(d + 1) * P],
                                             ident_sb[:], is_transpose=True,
                                             start=(k == 0), stop=(k == 3),
                                             skip_group_check=True)
                        xgt = xtpool.tile([P, NO], BF16, tag=f"xg{tci}_{g}",
                                          name=f"xg{tci}_{g}")
                        nc.vector.tensor_copy(xgt[:], tp[:].bitcast(F32))
                        xg[tci][g] = xgt
                return xg

            xg_pending = {sp: emit_x_transposes(sp, x_nat_pending.pop(sp))
                          for sp in range(2)}

            # ---- bias broadcast across partitions: [128, 3072] ----
            bias_bc = cpool.tile([P, O], F32, tag="biasbc")
            for j in range(N_OC):
                sl = slice(j * NO, (j + 1) * NO)
                b_ps = psA.tile([P, NO], F32, tag="acc")
                nc.tensor.matmul(b_ps[:], ones_sb[:], bias1_sb[:, sl],
                                 start=True, stop=True)
                nc.vector.tensor_copy(bias_bc[:, sl], b_ps[:])

            # ---- resident W'^T, 48 bf16 tiles [128, 512]: wt[d][ocb] ----
            wt = [[wpool.tile([P, NO], BF16, tag=f"wt{d}_{ocb}",
                              name=f"wt{d}_{ocb}")
                   for ocb in range(N_OC)] for d in range(N_D)]

            def emit_w_prep_group(ocb, d):
                # one PSUM group: 4 PE transposes (+ LoRA accumulate) + drain
                tp = psT.tile([P, NO], F32R, tag="tp")
                for j in range(TC):
                    nc.tensor.matmul(tp[:, j * P:(j + 1) * P],
                                     w_nat[ocb][j][:, d * P:(d + 1) * P],
                                     ident_sb[:], is_transpose=True,
                                     start=(j == 0),
                                     stop=(j == TC - 1 and ocb < 2),
                                     skip_group_check=True)
                if ocb >= 2:
                    f = 0 if ocb < 4 else 1
                    lo = ocb * NO - D - (D if f else 0)
                    nc.tensor.matmul(tp[:].bitcast(F32),
                                     a_sb[f][:, d * P:(d + 1) * P],
                                     bt_sb[f][:, lo:lo + NO],
                                     start=False, stop=True,
                                     skip_group_check=True)
                nc.vector.tensor_copy(wt[d][ocb][:], tp[:].bitcast(F32))

            # ---- one accumulation group of main matmuls + drain + store ----
            def emit_acc_group(sp, tci, oc, xg):
                trow = slice(sp * NO + tci * P, sp * NO + (tci + 1) * P)
                osl = slice(oc * NO, (oc + 1) * NO)
                acc = psA.tile([P, NO], F32, tag="acc", name="acc")
                for d in range(N_D):
                    lhsT = xg[tci][d // 4][:, (d % 4) * P:(d % 4 + 1) * P]
                    nc.tensor.matmul(acc[:], lhsT, wt[d][oc][:],
                                     start=(d == 0), stop=(d == N_D - 1))
                o_sb = opool.tile([P, NO], F32, tag="ost", name="ost")
                nc.vector.tensor_tensor(out=o_sb[:], in0=acc[:],
                                        in1=bias_bc[:, osl], op=ADD)
                nc.sync.dma_start(out=out_d[trow, osl], in_=o_sb[:])

            # ---- startup: W prep interleaved with sp0/sp1 matmuls ----
            # prep(0) first, then for each ocb: 8 acc groups (sp0/sp1 x 4 tci)
            # interleaved 1:1 with the 8 prep groups of ocb+1.
            for d in range(N_D):
                emit_w_prep_group(0, d)
            for ocb in range(N_OC):
                groups = [(sp, tci) for sp in (0, 1) for tci in range(TC)]
                for i, (sp, tci) in enumerate(groups):
                    emit_acc_group(sp, tci, ocb, xg_pending[sp])
                    if ocb + 1 < N_OC:
                        emit_w_prep_group(ocb + 1, i)

            # ---- steady state: superchunks 2..7 ----
            for sp in range(2, N_SUP):
                x_nat = emit_x_loads(sp)
                xg = emit_x_transposes(sp, x_nat)
                for tci in range(TC):
                    for oc in range(N_OC):
                        emit_acc_group(sp, tci, oc, xg)

    nc.compile()
    return nc


def kernel(x, W, bias, A0, A1, B0, B1, s0, s1, **run_kwargs):
    x = np.asarray(x, dtype=np.float32)
    if "nc" not in _CACHE:
        _CACHE["nc"] = _build()
    nc = _CACHE["nc"]

    s0 = np.float32(np.asarray(s0).reshape(()))
    s1 = np.float32(np.asarray(s1).reshape(()))
    shared = {
        "w": np.ascontiguousarray(np.asarray(W, np.float32)),
        "bias": np.asarray(bias, np.float32).reshape(1, O),
        "a0": np.ascontiguousarray(np.asarray(A0, np.float32)),
        "a1": np.ascontiguousarray(np.asarray(A1, np.float32)),
        "bt0": np.ascontiguousarray((s0 * np.asarray(B0, np.float32)).T),
        "bt1": np.ascontiguousarray((s1 * np.asarray(B1, np.float32)).T),
        "ident": np.eye(P, dtype=np.float32),
        "ones": np.ones((1, P), np.float32),
    }
    xr = x.reshape(N_CORES, TOK, D)
    in_maps = [{**shared, "x": np.ascontiguousarray(xr[c])} for c in range(N_CORES)]
    res = run_bass_kernel_spmd(nc, in_maps, list(range(N_CORES)), **run_kwargs)
    out = np.concatenate([res.results[c]["out"][None] for c in range(N_CORES)], 0)
    full = out.reshape(B, S, O)
    _CACHE["last_result"] = res
    return full
